# revision 6
# baseline (speedup 1.0000x reference)
"""Bass/Trainium2 kernel for nn_BiLSTM_9028021256417.

Reference computation (see problem): 2-layer "bidirectional" LSTM where the
fw and bw chains are independent (no concat between layers), residual add on
the last layer, final output = (fw + bw) / 2.

Sharding (8 NeuronCores, SPMD — identical program, per-core data):
  cores 0-3: forward direction,  batch shards of 128
  cores 4-7: backward direction, batch shards of 128 (host feeds
             time-reversed x, so the device program is direction-agnostic)

Device layout ("layout A"): all state kept transposed —
  h, c           : [H=128 partitions, B=128 free]
  PSUM gate bank : [128, 4*B] with gate order (g, f, i, o) along free dim
  per-gate matmul: out[128, B] (+)= lhsT(W_g|U_g [128,128]).T @ rhs(x_t^T|h)
Matmul inputs are bf16 (1 cycle/row on the PE; fp32 would be 4), PSUM
accumulation and the cell state c stay fp32.
"""

import numpy as np
import ml_dtypes

import concourse.bass as bass
import concourse.tile as tile
from concourse import bacc, mybir
from concourse.bass_utils import run_bass_kernel_spmd

AF = mybir.ActivationFunctionType
FP32 = mybir.dt.float32
BF16 = mybir.dt.bfloat16
NP_BF16 = ml_dtypes.bfloat16

# Problem sizes (hardcoded per the harness contract).
B_TOT, T, E, H = 512, 200, 128, 128
NCORES = 8
NSHARD = 4          # batch shards per direction
B = B_TOT // NSHARD  # 128 per core
P = 128
NG = 4

# Device gate order (g, f, i, o) -> Keras 4H order is (i, f, g, o).
# keras slice index for each device gate slot:
KERAS_IDX = [2, 1, 0, 3]  # g, f, i, o
COL_G = slice(0 * B, 1 * B)
COL_F = slice(1 * B, 2 * B)
COL_I = slice(2 * B, 3 * B)
COL_O = slice(3 * B, 4 * B)
COL_FI = slice(1 * B, 3 * B)


def _build_program(scalar_bias: float | None, t_steps: int = T):
    """Build the SPMD per-core Bass program.

    scalar_bias: if not None, every gate bias is this constant (baked as an
    ACT immediate and the f,i sigmoids fuse into one instruction). If None,
    biases are loaded from the "bias" DRAM input as per-partition [128,1]
    APs, and f/i sigmoids are split (bias differs per gate).
    """
    nc = bacc.Bacc("TRN2", target_bir_lowering=False, debug=False)

    xT = nc.dram_tensor("xT", [t_steps, E, B], BF16, kind="ExternalInput").ap()
    w = nc.dram_tensor("w", [2, NG, P, P], BF16, kind="ExternalInput").ap()
    u = nc.dram_tensor("u", [2, NG, P, P], BF16, kind="ExternalInput").ap()
    bias = nc.dram_tensor("bias", [2, NG, P, 1], FP32, kind="ExternalInput").ap()
    out = nc.dram_tensor("out", [t_steps, H, B], FP32, kind="ExternalOutput").ap()

    with tile.TileContext(nc) as tc:
        with (
            tc.tile_pool(name="wpool", bufs=1) as wpool,
            tc.tile_pool(name="xpool", bufs=16) as xpool,
            tc.tile_pool(name="zpool", bufs=2, space="PSUM") as zpool,
            tc.tile_pool(name="gpool", bufs=3) as gpool,
            tc.tile_pool(name="tpool", bufs=3) as tpool,
            tc.tile_pool(name="cpool", bufs=3) as cpool,
            tc.tile_pool(name="hpool", bufs=3) as hpool,
            tc.tile_pool(name="opool", bufs=4) as opool,
        ):
            w_t: dict = {}
            u_t: dict = {}
            b_t: dict = {}
            for l in range(2):
                for g in range(NG):
                    wt = wpool.tile([P, P], BF16, tag=f"w{l}{g}")
                    nc.sync.dma_start(wt[:], w[l, g])
                    w_t[l, g] = wt
                    ut = wpool.tile([P, P], BF16, tag=f"u{l}{g}")
                    nc.sync.dma_start(ut[:], u[l, g])
                    u_t[l, g] = ut
                    if scalar_bias is None:
                        bt = wpool.tile([P, 1], FP32, tag=f"b{l}{g}")
                        nc.sync.dma_start(bt[:], bias[l, g])
                        b_t[l, g] = bt

            def bias_arg(l, g):
                if scalar_bias is not None:
                    return scalar_bias
                return b_t[l, g][:]

            def cell(l, z, c_prev):
                """Gate activations + cell update for one layer-step.

                z: PSUM [128, 4B] fp32 (pre-activation gates, order g,f,i,o).
                Returns (h_new bf16 [128,B], c_new fp32 [128,B]).
                """
                tg = gpool.tile([P, B], BF16, tag=f"tg{l}")
                nc.scalar.activation(tg[:], z[:, COL_G], AF.Tanh, bias=bias_arg(l, 0))
                fi = gpool.tile([P, 2 * B], BF16, tag=f"fi{l}")
                if scalar_bias is not None:
                    nc.scalar.activation(
                        fi[:], z[:, COL_FI], AF.Sigmoid, bias=scalar_bias
                    )
                else:
                    nc.scalar.activation(
                        fi[:, 0:B], z[:, COL_F], AF.Sigmoid, bias=bias_arg(l, 1)
                    )
                    nc.scalar.activation(
                        fi[:, B : 2 * B], z[:, COL_I], AF.Sigmoid, bias=bias_arg(l, 2)
                    )
                og = gpool.tile([P, B], BF16, tag=f"o{l}")
                nc.scalar.activation(og[:], z[:, COL_O], AF.Sigmoid, bias=bias_arg(l, 3))

                t1 = tpool.tile([P, B], FP32, tag=f"t1{l}")
                nc.vector.tensor_mul(t1[:], fi[:, B : 2 * B], tg[:])  # i * g
                if c_prev is None:
                    c_new = t1
                else:
                    t2 = tpool.tile([P, B], FP32, tag=f"t2{l}")
                    nc.vector.tensor_mul(t2[:], fi[:, 0:B], c_prev[:])  # f * c
                    c_new = cpool.tile([P, B], FP32, tag=f"c{l}")
                    nc.vector.tensor_add(c_new[:], t1[:], t2[:])
                tch = gpool.tile([P, B], BF16, tag=f"tc{l}")
                nc.scalar.activation(tch[:], c_new[:], AF.Tanh)
                h_new = hpool.tile([P, B], BF16, tag=f"h{l}")
                nc.vector.tensor_mul(h_new[:], og[:], tch[:])
                return h_new, c_new

            h = {0: None, 1: None}
            c = {0: None, 1: None}
            for t in range(t_steps):
                xt = xpool.tile([P, B], BF16, tag="xt")
                nc.sync.dma_start(xt[:], xT[t])

                # ---- layer 0 ----
                z0 = zpool.tile([P, NG * B], FP32, tag="z0")
                first = h[0] is None
                for g in range(NG):
                    nc.tensor.matmul(
                        z0[:, g * B : (g + 1) * B],
                        lhsT=w_t[0, g][:],
                        rhs=xt[:],
                        start=(g == 0),
                        stop=(first and g == NG - 1),
                    )
                if not first:
                    for g in range(NG):
                        nc.tensor.matmul(
                            z0[:, g * B : (g + 1) * B],
                            lhsT=u_t[0, g][:],
                            rhs=h[0][:],
                            start=False,
                            stop=(g == NG - 1),
                        )
                h0, c0 = cell(0, z0, c[0])

                # ---- layer 1 (input = h0 of this step) ----
                z1 = zpool.tile([P, NG * B], FP32, tag="z1")
                first = h[1] is None
                for g in range(NG):
                    nc.tensor.matmul(
                        z1[:, g * B : (g + 1) * B],
                        lhsT=w_t[1, g][:],
                        rhs=h0[:],
                        start=(g == 0),
                        stop=(first and g == NG - 1),
                    )
                if not first:
                    for g in range(NG):
                        nc.tensor.matmul(
                            z1[:, g * B : (g + 1) * B],
                            lhsT=u_t[1, g][:],
                            rhs=h[1][:],
                            start=False,
                            stop=(g == NG - 1),
                        )
                h1, c1 = cell(1, z1, c[1])

                # residual (last layer): out_t = h1 + h0
                ot = opool.tile([P, B], FP32, tag="ot")
                nc.vector.tensor_add(ot[:], h1[:], h0[:])
                nc.sync.dma_start(out[t], ot[:])

                h[0], c[0], h[1], c[1] = h0, c0, h1, c1

    nc.compile()
    return nc


_PROGRAM_CACHE: dict = {}


def _get_program(scalar_bias, t_steps: int = T):
    key = (scalar_bias, t_steps)
    if key not in _PROGRAM_CACHE:
        _PROGRAM_CACHE[key] = _build_program(scalar_bias, t_steps)
    return _PROGRAM_CACHE[key]


def _prep_inputs(x, W, U, b, scalar_bias):
    """Build the 8 per-core input maps."""
    in_maps = []
    per_dir = {}
    for d in range(2):
        wd = np.empty((2, NG, P, P), dtype=NP_BF16)
        ud = np.empty((2, NG, P, P), dtype=NP_BF16)
        bd = np.empty((2, NG, P, 1), dtype=np.float32)
        for l in range(2):
            for g in range(NG):
                ks = KERAS_IDX[g]
                wd[l, g] = W[l, d][:, ks * H : (ks + 1) * H].astype(NP_BF16)
                ud[l, g] = U[l, d][:, ks * H : (ks + 1) * H].astype(NP_BF16)
                bd[l, g, :, 0] = b[l, d][ks * H : (ks + 1) * H].astype(np.float32)
        per_dir[d] = (wd, ud, bd)

    for core in range(NCORES):
        d = core // NSHARD
        s = core % NSHARD
        xs = x[s * B : (s + 1) * B]           # [B, T, E]
        if d == 1:
            xs = xs[:, ::-1, :]               # time-reverse for backward dir
        xTc = np.ascontiguousarray(np.transpose(xs, (1, 2, 0))).astype(NP_BF16)
        wd, ud, bd = per_dir[d]
        in_maps.append({"xT": xTc, "w": wd, "u": ud, "bias": bd})
    return in_maps


def _postprocess(results, dtype):
    full = np.empty((B_TOT, T, H), dtype=np.float32)
    for s in range(NSHARD):
        fw = np.asarray(results[s]["out"])            # [T, H, B]
        bw = np.asarray(results[NSHARD + s]["out"])   # [T, H, B] (reversed time)
        fw_b = np.transpose(fw, (2, 0, 1))            # [B, T, H]
        bw_b = np.transpose(bw, (2, 0, 1))[:, ::-1, :]
        full[s * B : (s + 1) * B] = (fw_b + bw_b) * 0.5
    return full.astype(dtype)


def run(x, W, U, b, **spmd_kwargs):
    """Run the kernel; returns (output, BassKernelResults)."""
    x = np.asarray(x)
    W = np.asarray(W)
    U = np.asarray(U)
    b = np.asarray(b)
    b0 = float(np.asarray(b).flat[0])
    scalar_bias = b0 if np.all(b == b0) else None
    nc = _get_program(scalar_bias)
    in_maps = _prep_inputs(x, W, U, b, scalar_bias)
    res = run_bass_kernel_spmd(nc, in_maps, core_ids=list(range(NCORES)), **spmd_kwargs)
    out = _postprocess(res.results, x.dtype)
    return out, res


def kernel(x, W, U, b):
    out, _ = run(x, W, U, b)
    return out
